# revision 13
# baseline (speedup 1.0000x reference)
"""GNN message-passing layer for Trainium2, SPMD over 8 NeuronCores.

Computes, per batch b:
    XI = x @ Wi + b_msg                  # [N, F]
    XJ = x @ Wj                          # [N, F]
    agg[i, o] = sum_j adj[i, j] * relu(XI[i, o] + XJ[j, o])
    out = relu(x @ Wu1 + agg @ Wu2 + b_upd)

Sharding: core c handles batch b = c // 2 and i-rows [ih*256, ih*256+256),
ih = c % 2.  Each core gets full x (XJ needs all j), its i-slice of x and
adj, and replicated weights; outputs are the core's [256, 128] out-slice.

Per-core schedule (messages in bf16):
  - XI rows packed to one-partition-per-group layout; GPSIMD
    partition_broadcast materializes xib[j, (i,o)] per group of G i's.
  - DVE scalar_tensor_tensor adds XJ (free-dim step-0 repeat over i) + xib.
  - Relu split between DVE tensor_scalar (4x bf16) and ACT activation.
  - PE reduces over j: per (i, jt) matmul with relu'd messages [j, o] as
    stationary and the adj column [j, 1] as 1-wide moving operand,
    accumulating aggT[o, i] columns in PSUM across the 4 j-tiles.
  - Final linear fused as two f32 matmuls into one PSUM tile + ACT relu,
    PE transposes, DMA out.
"""

import numpy as np
from contextlib import ExitStack

import concourse.bass as bass
import concourse.mybir as mybir
import concourse.tile as tile
from concourse import bacc
from concourse.bass import ts
from concourse.bass_utils import run_bass_kernel_spmd
from concourse.masks import make_identity

B, N, F = 4, 512, 128
NCORES = 8
P = 128
IH = N // 2            # i-rows per core
NJT = N // P           # 4 j-tiles
NIT = IH // P          # 2 i-tiles
G = 16                 # i-group size for broadcast batching
NG = IH // G           # 16 groups
GF = G * F             # free size of one batched message tile
MSG_DT = mybir.dt.bfloat16
F32 = mybir.dt.float32
# fraction of relu instructions routed to ACT (rest on DVE tensor_scalar)
ACT_RELU_NUM, ACT_RELU_DEN = 3, 5


def _kernel_body(ctx: ExitStack, tc: tile.TileContext, x, xh, adjh, w_msg,
                 b_msg, w_upd, b_upd, out, reps=1):
    nc = tc.nc
    RELU = mybir.ActivationFunctionType.Relu

    singles = ctx.enter_context(tc.tile_pool(name="singles", bufs=1))
    loads = ctx.enter_context(tc.tile_pool(name="loads", bufs=1))
    mpool = ctx.enter_context(tc.tile_pool(name="mpool", bufs=3))
    rpool = ctx.enter_context(tc.tile_pool(name="rpool", bufs=6))
    xibp = ctx.enter_context(tc.tile_pool(name="xibp", bufs=2))
    opool = ctx.enter_context(tc.tile_pool(name="opool", bufs=2))
    ppool = ctx.enter_context(tc.tile_pool(name="ppool", bufs=2, space="PSUM"))
    pagg = ctx.enter_context(tc.tile_pool(name="pagg", bufs=1, space="PSUM"))

    # ---- constants / weights -------------------------------------------
    identity = singles.tile([P, P], F32)
    make_identity(nc, identity)
    ones1 = singles.tile([1, P], F32)
    nc.vector.memset(ones1, 1.0)

    wi_sb = singles.tile([P, F], F32)
    nc.sync.dma_start(out=wi_sb, in_=w_msg[0:F, :])
    wj_sb = singles.tile([P, F], F32)
    nc.sync.dma_start(out=wj_sb, in_=w_msg[F:2 * F, :])
    wu1_sb = singles.tile([P, F], F32)
    nc.sync.dma_start(out=wu1_sb, in_=w_upd[0:F, :])
    wu2_sb = singles.tile([P, F], F32)
    nc.sync.dma_start(out=wu2_sb, in_=w_upd[F:2 * F, :])
    bmsg_sb = singles.tile([1, F], F32)
    nc.sync.dma_start(out=bmsg_sb, in_=b_msg[:, :])
    bupd_sb = singles.tile([P, 1], F32)
    nc.sync.dma_start(out=bupd_sb, in_=b_upd[:, :])

    # ---- load x / xh / adjh --------------------------------------------
    x_sb = loads.tile([P, N // P, F], F32)
    nc.sync.dma_start(out=x_sb, in_=x.rearrange("(t p) f -> p t f", p=P))
    xh_sb = loads.tile([P, NIT, F], F32)
    nc.sync.dma_start(out=xh_sb, in_=xh.rearrange("(t p) f -> p t f", p=P))
    adjh_sb = loads.tile([P, NIT, N], F32)
    nc.sync.dma_start(out=adjh_sb, in_=adjh.rearrange("(t p) j -> p t j", p=P))

    # ---- transposes: xbT [f, n=512], xhT [f, i=256] --------------------
    xbT = singles.tile([P, N], F32)
    for t in range(N // P):
        ps = ppool.tile([P, P], F32, tag="tp")
        nc.tensor.transpose(ps[:], x_sb[:, t, :], identity[:])
        nc.scalar.copy(out=xbT[:, ts(t, P)], in_=ps[:])
    xhT = singles.tile([P, IH], F32)
    for t in range(NIT):
        ps = ppool.tile([P, P], F32, tag="tp")
        nc.tensor.transpose(ps[:], xh_sb[:, t, :], identity[:])
        nc.scalar.copy(out=xhT[:, ts(t, P)], in_=ps[:])

    # ---- adjT [j, (jt, i)] bf16 ----------------------------------------
    adjT = singles.tile([P, NJT, IH], MSG_DT)
    for it in range(NIT):
        for jt in range(NJT):
            ps = ppool.tile([P, P], F32, tag="tp")
            nc.tensor.transpose(ps[:], adjh_sb[:, it, ts(jt, P)], identity[:])
            nc.scalar.copy(out=adjT[:, jt, ts(it, P)], in_=ps[:])

    # ---- XJ [j, o] bf16 tiles; XI [i, o] bf16 --------------------------
    xj_sb = singles.tile([P, NJT, F], MSG_DT)
    for t in range(NJT):
        ps = ppool.tile([P, F], F32, tag="mm")
        nc.tensor.matmul(ps[:], lhsT=xbT[:, ts(t, P)], rhs=wj_sb[:],
                         start=True, stop=True)
        nc.scalar.copy(out=xj_sb[:, t, :], in_=ps[:])
    xi_sb = singles.tile([P, NIT, F], MSG_DT)
    for t in range(NIT):
        ps = ppool.tile([P, F], F32, tag="mm")
        nc.tensor.matmul(ps[:], lhsT=xhT[:, ts(t, P)], rhs=wi_sb[:],
                         start=True, stop=False)
        nc.tensor.matmul(ps[:], lhsT=ones1[0:1, :], rhs=bmsg_sb[0:1, :],
                         start=False, stop=True)
        nc.scalar.copy(out=xi_sb[:, t, :], in_=ps[:])

    # ---- pack XI rows into per-group partitions: xi_lay[g, (s, o)] -----
    xi_lay = singles.tile([1, NG * GF], MSG_DT)
    xi_lay3 = xi_lay[:].rearrange("p (g s f) -> p g s f", g=NG, f=F)
    gpp = P // G  # groups per source i-tile
    for t in range(NIT):
        for gg in range(gpp):
            g = t * gpp + gg
            nc.sync.dma_start(
                out=xi_lay3[0:1, g, :, :],
                in_=xi_sb[gg * G:(gg + 1) * G, t, :],
            )

    # ---- main loop ------------------------------------------------------
    for _rep in range(reps):
        _main_loop(nc, tc, mpool, rpool, xibp, opool, ppool, pagg, xi_lay,
                   xj_sb, adjT, xhT, wu1_sb, wu2_sb, bupd_sb, identity, out)


def _main_loop(nc, tc, mpool, rpool, xibp, opool, ppool, pagg, xi_lay,
               xj_sb, adjT, xhT, wu1_sb, wu2_sb, bupd_sb, identity, out):
    RELU = mybir.ActivationFunctionType.Relu
    paggT = pagg.tile([P, IH], F32)   # aggT[o, i] accumulator
    k = 0
    for g in range(NG):
        xib = xibp.tile([P, GF], MSG_DT)
        nc.gpsimd.partition_broadcast(xib[:, :],
                                      xi_lay[0:1, g * GF:(g + 1) * GF],
                                      channels=P)
        xib3 = xib[:].rearrange("p (s f) -> p s f", f=F)
        mrelus = []
        for jt in range(NJT):
            xj_ap = xj_sb[:, jt, :]
            xj_rep = bass.AP(tensor=xj_ap.tensor, offset=xj_ap.offset,
                             ap=[xj_ap.ap[0], [0, G], xj_ap.ap[1]])
            msum = mpool.tile([P, G, F], MSG_DT)
            nc.vector.scalar_tensor_tensor(
                out=msum[:, :, :], in0=xj_rep, scalar=0.0, in1=xib3,
                op0=mybir.AluOpType.add, op1=mybir.AluOpType.add)
            mrelu = rpool.tile([P, G, F], MSG_DT)
            if (k * ACT_RELU_NUM) % ACT_RELU_DEN < ACT_RELU_NUM:
                nc.scalar.activation(mrelu[:, :, :], msum[:, :, :], RELU)
            else:
                nc.vector.tensor_scalar_max(mrelu[:, :, :], msum[:, :, :],
                                            0.0)
            k += 1
            mrelus.append(mrelu)
        for s in range(G):
            iloc = g * G + s
            for jt in range(NJT):
                nc.tensor.matmul(
                    paggT[:, iloc:iloc + 1],
                    lhsT=mrelus[jt][:, s, :],
                    rhs=adjT[:, jt, iloc:iloc + 1],
                    start=(jt == 0), stop=(jt == NJT - 1))

    # ---- epilogue: z = relu(x@Wu1 + agg@Wu2 + b_upd) -------------------
    aggT_sb = opool.tile([P, IH], F32)
    nc.scalar.copy(out=aggT_sb[:, :], in_=paggT[:, :])
    pz = pagg.tile([P, IH], F32)
    nc.tensor.matmul(pz[:], lhsT=wu1_sb[:], rhs=xhT[:, :],
                     start=True, stop=False)
    nc.tensor.matmul(pz[:], lhsT=wu2_sb[:], rhs=aggT_sb[:, :],
                     start=False, stop=True)
    zr = opool.tile([P, IH], F32)
    nc.scalar.activation(zr[:, :], pz[:, :], RELU, bias=bupd_sb[:, 0:1])

    out_sb = opool.tile([P, NIT, F], F32)
    for it in range(NIT):
        ps = ppool.tile([P, P], F32, tag="tp")
        nc.tensor.transpose(ps[:], zr[:, ts(it, P)], identity[:])
        nc.scalar.copy(out=out_sb[:, it, :], in_=ps[:])
    nc.sync.dma_start(out=out.rearrange("(t p) f -> p t f", p=P), in_=out_sb)


def build_nc(reps=1) -> bass.Bass:
    nc = bacc.Bacc("TRN2", target_bir_lowering=False, debug=False,
                   num_devices=NCORES)
    x = nc.dram_tensor("x", [N, F], F32, kind="ExternalInput")
    xh = nc.dram_tensor("xh", [IH, F], F32, kind="ExternalInput")
    adjh = nc.dram_tensor("adjh", [IH, N], F32, kind="ExternalInput")
    w_msg = nc.dram_tensor("w_msg", [2 * F, F], F32, kind="ExternalInput")
    b_msg = nc.dram_tensor("b_msg", [1, F], F32, kind="ExternalInput")
    w_upd = nc.dram_tensor("w_upd", [2 * F, F], F32, kind="ExternalInput")
    b_upd = nc.dram_tensor("b_upd", [F, 1], F32, kind="ExternalInput")
    out = nc.dram_tensor("out", [IH, F], F32, kind="ExternalOutput")
    with tile.TileContext(nc) as tc, ExitStack() as ctx:
        _kernel_body(ctx, tc, x[:], xh[:], adjh[:], w_msg[:], b_msg[:],
                     w_upd[:], b_upd[:], out[:], reps=reps)
    nc.compile()
    return nc


def make_in_maps(x, adj, W_msg, b_msg, W_upd, b_upd):
    in_maps = []
    for c in range(NCORES):
        b, ih = c // 2, c % 2
        sl = slice(ih * IH, (ih + 1) * IH)
        in_maps.append({
            "x": np.ascontiguousarray(x[b]),
            "xh": np.ascontiguousarray(x[b, sl]),
            "adjh": np.ascontiguousarray(adj[b, sl]),
            "w_msg": np.ascontiguousarray(W_msg),
            "b_msg": np.ascontiguousarray(b_msg.reshape(1, F)),
            "w_upd": np.ascontiguousarray(W_upd),
            "b_upd": np.ascontiguousarray(b_upd.reshape(F, 1)),
        })
    return in_maps


_NC_CACHE = None


def kernel(x, adj, W_msg, b_msg, W_upd, b_upd, _trace=False):
    global _NC_CACHE
    x = np.asarray(x, dtype=np.float32)
    adj = np.asarray(adj, dtype=np.float32)
    in_maps = make_in_maps(x, adj, np.asarray(W_msg, np.float32),
                           np.asarray(b_msg, np.float32),
                           np.asarray(W_upd, np.float32),
                           np.asarray(b_upd, np.float32))
    if _NC_CACHE is None:
        _NC_CACHE = build_nc()
    res = run_bass_kernel_spmd(_NC_CACHE, in_maps,
                               core_ids=list(range(NCORES)), trace=_trace)
    out = np.empty((B, N, F), dtype=np.float32)
    for c in range(NCORES):
        b, ih = c // 2, c % 2
        out[b, ih * IH:(ih + 1) * IH] = res.results[c]["out"]
    if _trace:
        kernel.last_results = res
    return out
